# revision 32
# baseline (speedup 1.0000x reference)
"""Trainium2 Bass kernel for masked BasicBlock (grouped conv3x3 -> BN -> ReLU
-> masked grouped conv3x3 -> BN -> +residual -> ReLU).

Strategy: data-parallel over batch across 8 NeuronCores (2 images/core).
Grouped conv mapped to accumulating matmuls over a zero-padded SBUF image
layout with a row-duplicated ("dup") input so one K=128 matmul covers two ky
taps; the third ky row runs as K=64 matmuls on PE row-groups 2-3 read from the
shifted partition half.  Per 8-row output tile (N=448): 6 matmul "slots", each
two column-concurrent M=64 matmuls (g0 in PE cols 0:64, g1 in 64:128).

BN training-mode stats: per-tile channel sums come free from the PSUM
evacuation (scalar-engine Copy with accum_out); sums of squares from one DVE
tensor_tensor_reduce per tile.  Global stats via one 2KB AllReduce per conv.

Conv2's masked input is built on-device: DVE computes m2*relu(a1*c1+b1) into a
padded tile, and the dup layout is produced by SBUF->SBUF DMAs on otherwise
idle DMA engines.  Output is written bf16 and widened to f32 on host.

Self-contained: hardcodes shapes from the problem spec.
"""
from contextlib import ExitStack

import numpy as np
import ml_dtypes

import concourse.bacc as bacc
import concourse.bass as bass
import concourse.mybir as mybir
from concourse.tile import TileContext
from concourse.bass_utils import run_bass_kernel_spmd

F32 = mybir.dt.float32
BF16 = mybir.dt.bfloat16
AF = mybir.ActivationFunctionType
ALU = mybir.AluOpType

N_CORES = 8
IMG = 2              # images per core
CIN = 256
G = 4
PAIRS = 2            # pairs of channel groups (128 ch each)
H = W = 56
PH, PW = 59, 58      # padded rows / cols (rows 0,57,58 and cols 0,57 zero)
PADN = PH * PW       # 3422
INT0 = PW            # flat offset of padded row 1
INTN = 56 * PW       # 3248: rows 1..56, all 58 cols
ROWT = 7             # 8-row output tiles per image
TN = 8 * W           # 448 pixels per psum tile
EPS = 1e-5
N_TOT = 16 * H * W
HNW = H * W // 2     # 1568

_prog_cache = {}


def _sub_ap(base, off, dims):
    """Custom free-dim access pattern on an existing AP (keeps partition dim)."""
    return bass.AP(
        tensor=base.tensor,
        offset=base.offset + off,
        ap=[list(base.ap[0])] + [list(d) for d in dims],
    )


def _widx(conv, pair, g, dx):
    return ((conv * PAIRS + pair) * 2 + g) * 3 + dx


def _build_program():
    nc = bacc.Bacc(num_devices=N_CORES)

    # conv1 input: host-premasked, padded, row-dup layout; g0|g1 concat per pair
    xcc_d = nc.dram_tensor("xcc", [IMG, PAIRS, 128, 2 * PADN], BF16,
                           kind="ExternalInput")
    xr_d = nc.dram_tensor("xres", [IMG, PAIRS, 128, H * W], BF16,
                          kind="ExternalInput")
    y_d = nc.dram_tensor("y", [IMG, PAIRS, 128, H * W], BF16,
                         kind="ExternalOutput")
    # all conv weights: 24 pair-tap lhsT [128,64] then 24 ky2 lhsT (rows 64:128)
    wall_d = nc.dram_tensor("wall", [128, 48 * 64], BF16, kind="ExternalInput")
    mr_d = nc.dram_tensor("mrow", [128, IMG * PAIRS * 7 * PW], BF16,
                          kind="ExternalInput")
    gb_d = nc.dram_tensor("gb", [128, 8], F32, kind="ExternalInput")

    with TileContext(nc) as tc, ExitStack() as es:
        consts = es.enter_context(tc.tile_pool(name="consts", bufs=1))
        small = es.enter_context(tc.tile_pool(name="small", bufs=16))
        ccp = es.enter_context(tc.tile_pool(name="ccp", bufs=4))
        c2p = es.enter_context(tc.tile_pool(name="c2p", bufs=4))
        m2p = es.enter_context(tc.tile_pool(name="m2p", bufs=4))
        xrp = es.enter_context(tc.tile_pool(name="xrp", bufs=4))
        yp = es.enter_context(tc.tile_pool(name="yp", bufs=4))
        scrp = es.enter_context(tc.tile_pool(name="scrp", bufs=2))
        psp = es.enter_context(tc.tile_pool(name="psp", bufs=8, space="PSUM"))
        drp = es.enter_context(tc.tile_pool(name="drp", bufs=1, space="DRAM"))

        # ---- collectives firmware warmup AllReduce (tiny, issued first) ----
        ccw_in = drp.tile([128, 4], F32, tag="ccwin", name="ccwin")
        ccw_out = drp.tile([128, 4], F32, addr_space="Shared",
                           tag="ccwout", name="ccwout")
        ccw2_in = drp.tile([128, 4], F32, tag="ccw2in", name="ccw2in")
        ccw2_out = drp.tile([128, 4], F32, addr_space="Shared",
                            tag="ccw2out", name="ccw2out")
        warm = small.tile([128, 4], F32, tag="warm", name="warm")
        nc.vector.memset(warm[:], 0.0)
        nc.sync.dma_start(out=ccw_in[:], in_=warm[:])
        nc.sync.dma_start(out=ccw2_in[:], in_=warm[:])
        for wi, wo in ((ccw_in, ccw_out), (ccw2_in, ccw2_out)):
            nc.gpsimd.collective_compute(
                "AllReduce", ALU.add,
                replica_groups=[list(range(N_CORES))],
                ins=[wi[:]], outs=[wo[:]],
            )

        cc_in = {c: drp.tile([128, 4], F32, tag=f"ccin{c}", name=f"ccin{c}")
                 for c in range(2)}
        cc_out = {c: drp.tile([128, 4], F32, addr_space="Shared",
                              tag=f"ccout{c}", name=f"ccout{c}") for c in range(2)}

        # ---- constants to SBUF (batched DMAs) ----
        wall_sb = consts.tile([128, 48 * 64], BF16, tag="wall", name="wall")
        nc.sync.dma_start(out=wall_sb[:], in_=wall_d[:])
        mr_sb = consts.tile([128, IMG * PAIRS * 7 * PW], BF16, tag="mr", name="mr")
        nc.sync.dma_start(out=mr_sb[:], in_=mr_d[:])
        gb_sb = consts.tile([128, 8], F32, tag="gb", name="gb")
        nc.sync.dma_start(out=gb_sb[:], in_=gb_d[:])
        eps_sb = consts.tile([128, 1], F32, tag="eps", name="eps")
        nc.vector.memset(eps_sb[:], EPS)

        def wp_ap(conv, pair, g, dx):      # pair-tap lhsT [128, 64]
            i = _widx(conv, pair, g, dx) * 64
            return wall_sb[:, i:i + 64]

        def wk_ap(conv, pair, g, dx):      # ky2 lhsT [128, 64], rows 0:64 zero
            i = (24 + _widx(conv, pair, g, dx)) * 64
            return wall_sb[:, i:i + 64]

        craw = {}
        for pair in range(PAIRS):
            for img in range(IMG):
                craw[(pair, img)] = consts.tile(
                    [128, H * W], BF16, tag=f"cr{pair}{img}", name=f"cr{pair}{img}")

        # bn_stats output groups: 6 values per (img, tile)
        st = {(c, p): consts.tile([128, IMG * ROWT * 6], F32,
                                  tag=f"st{c}{p}", name=f"st{c}{p}")
              for c in range(2) for p in range(PAIRS)}
        a_sb = {c: consts.tile([128, PAIRS], F32, tag=f"a{c}", name=f"a{c}")
                for c in range(2)}
        b_sb = {c: consts.tile([128, PAIRS], F32, tag=f"b{c}", name=f"b{c}")
                for c in range(2)}

        # ---------------- matmul block for one (conv, img, pair) ----------------
        def mm_block(conv, img, pair, rhs_tile):
            """rhs_tile(g) -> (tile, base_off) giving the dup layout source."""
            for t in range(ROWT):
                # full-bank pitch (512 f32) so partition offsets decompose
                # exactly in the accumulation-group bookkeeping
                ps = psp.tile([128, 512], F32, tag="ps", name="ps")
                # open the accumulation group for the whole bank with a tiny
                # N=1 matmul into the spare column (~50ns, M=128 start)
                tile0, off0 = rhs_tile(0)
                nc.tensor.matmul(
                    ps[:, TN:TN + 1], wall_sb[:, 0:128],
                    _sub_ap(tile0[:], off0, [[1, 1]]),
                    start=True, stop=False)
                for dx in range(3):
                    for g in range(2):
                        tile_g, off_g = rhs_tile(g)
                        rhs = _sub_ap(tile_g[:], off_g + (8 * t) * PW + dx,
                                      [[PW, 8], [1, W]])
                        nc.tensor.matmul(
                            ps[64 * g:64 * (g + 1), 0:TN],
                            wp_ap(conv, pair, g, dx), rhs,
                            start=False, stop=False)
                for dx in range(3):
                    for g in range(2):
                        tile_g, off_g = rhs_tile(g)
                        rhs = _sub_ap(tile_g[:], off_g + (8 * t + 1) * PW + dx,
                                      [[PW, 8], [1, W]])
                        nc.tensor.matmul(
                            ps[64 * g:64 * (g + 1), 0:TN],
                            wk_ap(conv, pair, g, dx), rhs,
                            start=False, stop=False)
                # close the group across all 128 partitions (tiny M=128 N=1)
                nc.tensor.matmul(
                    ps[:, TN:TN + 1], wall_sb[:, 0:128],
                    _sub_ap(tile0[:], off0, [[1, 1]]),
                    start=False, stop=True)
                # evacuate and take per-tile BN stats (baseline-proven path)
                seg = craw[(pair, img)][:, TN * t:TN * (t + 1)]
                col = img * ROWT + t
                nc.scalar.activation(out=seg, in_=ps[:, 0:TN], func=AF.Copy)
                nc.vector.bn_stats(
                    out=st[(conv, pair)][:, 6 * col:6 * (col + 1)], in_=seg)

        # ---------------- global BN stats -> a, b ----------------
        N_CORE_CNT = IMG * H * W

        def bn_coeffs(conv):
            sq = small.tile([128, 4], F32, tag=f"sq{conv}", name=f"sq{conv}")
            for pair in range(PAIRS):
                mv = small.tile([128, 2], F32, tag="mv", name="mv")
                nc.vector.bn_aggr(
                    out=mv[:],
                    in_=st[(conv, pair)][:].rearrange("p (n s) -> p n s", s=6))
                nc.vector.tensor_scalar_mul(
                    sq[:, 2 * pair:2 * pair + 1], mv[:, 0:1], float(N_CORE_CNT))
                msq0 = small.tile([128, 1], F32, tag="msq0", name="msq0")
                nc.vector.tensor_mul(msq0[:], mv[:, 0:1], mv[:, 0:1])
                nc.vector.tensor_add(msq0[:], msq0[:], mv[:, 1:2])
                nc.vector.tensor_scalar_mul(
                    sq[:, 2 * pair + 1:2 * pair + 2], msq0[:], float(N_CORE_CNT))
            nc.sync.dma_start(out=cc_in[conv][:], in_=sq[:])
            nc.gpsimd.collective_compute(
                "AllReduce", ALU.add,
                replica_groups=[list(range(N_CORES))],
                ins=[cc_in[conv][:]], outs=[cc_out[conv][:]],
            )
            sq2 = small.tile([128, 4], F32, tag=f"sq2{conv}", name=f"sq2{conv}")
            nc.sync.dma_start(out=sq2[:], in_=cc_out[conv][:])
            # batched over pairs: columns 0,2 are sums; 1,3 sum-squares
            mu = small.tile([128, PAIRS], F32, tag="mu", name="mu")
            nc.vector.tensor_scalar(
                out=mu[:], in0=_sub_ap(sq2[:], 0, [[2, PAIRS]]),
                scalar1=1.0 / N_TOT, scalar2=None, op0=ALU.mult)
            var = small.tile([128, PAIRS], F32, tag="var", name="var")
            nc.vector.tensor_scalar(
                out=var[:], in0=_sub_ap(sq2[:], 1, [[2, PAIRS]]),
                scalar1=1.0 / N_TOT, scalar2=None, op0=ALU.mult)
            msq = small.tile([128, PAIRS], F32, tag="msq", name="msq")
            nc.vector.tensor_mul(msq[:], mu[:], mu[:])
            nc.vector.tensor_sub(var[:], var[:], msq[:])       # biased var
            sd = small.tile([128, PAIRS], F32, tag="sd", name="sd")
            nc.scalar.activation(out=sd[:], in_=var[:], func=AF.Sqrt,
                                 bias=eps_sb[:])
            rstd = small.tile([128, PAIRS], F32, tag="rstd", name="rstd")
            nc.vector.reciprocal(out=rstd[:], in_=sd[:])
            gam = gb_sb[:, 4 * conv:4 * conv + 2]
            bet = gb_sb[:, 4 * conv + 2:4 * conv + 4]
            nc.vector.tensor_mul(a_sb[conv][:], gam, rstd[:])
            t3 = small.tile([128, PAIRS], F32, tag="t3", name="t3")
            nc.vector.tensor_mul(t3[:], a_sb[conv][:], mu[:])
            nc.vector.tensor_sub(b_sb[conv][:], bet, t3[:])

        # ---------------- conv1 ----------------
        cc_tiles = {}
        for img in range(IMG):
            for pair in range(PAIRS):
                cc = ccp.tile([128, 2 * PADN], BF16, tag="cc", name="cc")
                nc.sync.dma_start(out=cc[:], in_=xcc_d[img, pair])
                cc_tiles[(img, pair)] = cc
        for img in range(IMG):
            for pair in range(PAIRS):
                cc = cc_tiles[(img, pair)]
                mm_block(0, img, pair, lambda g, cc=cc: (cc, g * PADN))

        bn_coeffs(0)

        # ---------------- conv2 ----------------
        xr_tiles = {}
        for img in range(IMG):
            for pair in range(PAIRS):
                xr_tiles[(img, pair)] = xrp.tile([128, H * W], BF16,
                                                 tag="xr", name="xr")

        def conv2_block(img, pair):
            m2 = m2p.tile([128, PADN], BF16, tag="m2", name="m2")
            # zero borders: row 0, rows 57-58, cols 0 and 57 of rows 1-56
            nc.vector.memset(m2[:, 0:PW], 0)
            nc.vector.memset(m2[:, 57 * PW:PADN], 0)
            nc.vector.memset(_sub_ap(m2[:], PW, [[PW, 56], [1, 1]]), 0)
            nc.vector.memset(_sub_ap(m2[:], PW + 57, [[PW, 56], [1, 1]]), 0)
            # interior: relu(a1*c1 + b1) in one ACT op, then mask on DVE
            nc.scalar.activation(
                out=_sub_ap(m2[:], PW + 1, [[PW, 56], [1, 56]]),
                in_=craw[(pair, img)][:], func=AF.Relu,
                bias=b_sb[0][:, pair:pair + 1],
                scale=a_sb[0][:, pair:pair + 1])
            mask_ap = _sub_ap(mr_sb[:], (img * PAIRS + pair) * 7 * PW,
                              [[PW, 7], [0, 8], [1, PW]])
            nc.vector.tensor_mul(m2[:, INT0:INT0 + INTN],
                                 m2[:, INT0:INT0 + INTN], mask_ap)
            # dup layout via a DRAM round-trip: store m2, re-load each group
            # with a 3-dim source pattern that duplicates (rows | rows+1)
            m2d = drp.tile([128, PADN], BF16, tag="m2d", name="m2d")
            nc.sync.dma_start(out=m2d[0:64, :], in_=m2[0:64, :])
            nc.sync.dma_start(out=m2d[64:128, :], in_=m2[64:128, :])
            c2 = {}
            for g in range(2):
                c = c2p.tile([128, PADN], BF16, tag="c2", name="c2")
                half = m2d[64 * g:64 * (g + 1), :]
                nc.sync.dma_start(out=c[0:64, 0:PADN], in_=half)
                nc.sync.dma_start(out=c[64:128, 0:PADN - PW],
                                  in_=m2d[64 * g:64 * (g + 1), PW:PADN])
                c2[g] = c
            mm_block(1, img, pair, lambda g, c2=c2: (c2[g], 0))

        # residual loads: pinned to conv1 completion (sequencing write makes
        # the DMA wait for conv1's last bn_stats) so they fill the
        # collectives-bootstrap gap instead of contending with conv2 prep
        for img in range(IMG):
            for pair in range(PAIRS):
                xr = xr_tiles[(img, pair)]
                nc.vector.tensor_copy(out=xr[0:1, 0:1],
                                      in_=st[(0, 1)][0:1, 0:1])
                nc.sync.dma_start(out=xr[:], in_=xr_d[img, pair])

        conv2_block(0, 0)
        conv2_block(0, 1)
        conv2_block(1, 0)
        conv2_block(1, 1)

        bn_coeffs(1)

        # ---------------- final: relu(a2*c2 + b2 + x) -> y (bf16) ----------------
        for img in range(IMG):
            for pair in range(PAIRS):
                QNW = HNW // 2
                for q in range(4):
                    seg = slice(QNW * q, QNW * (q + 1))
                    yt = yp.tile([128, QNW], BF16, tag="yt", name="yt")
                    # u = a2*c2 + x on DVE; relu(u + b2) on the idle ACT
                    # engine so the two stages pipeline across chunks
                    nc.vector.scalar_tensor_tensor(
                        out=yt[:], in0=craw[(pair, img)][:, seg],
                        scalar=a_sb[1][:, pair:pair + 1],
                        in1=xr_tiles[(img, pair)][:, seg],
                        op0=ALU.mult, op1=ALU.add)
                    nc.scalar.activation(
                        out=yt[:], in_=yt[:], func=AF.Relu,
                        bias=b_sb[1][:, pair:pair + 1])
                    nc.sync.dma_start(
                        out=_sub_ap(y_d[img, pair], QNW * q, [[1, QNW]]),
                        in_=yt[:])

    nc.compile()
    return nc


def _pack_weights(w1, w2):
    """w [256,64,3,3] f32 x2 -> wall [128, 48*64] bf16."""
    wp = np.zeros([2, PAIRS, 2, 3, 128, 64], np.float32)
    wk = np.zeros([2, PAIRS, 2, 3, 128, 64], np.float32)
    for conv, w in enumerate([w1, w2]):
        for pair in range(PAIRS):
            for g in range(2):
                blk = w[64 * (2 * pair + g):64 * (2 * pair + g + 1)]
                for dx in range(3):
                    wp[conv, pair, g, dx, 0:64, :] = blk[:, :, 0, dx].T
                    wp[conv, pair, g, dx, 64:128, :] = blk[:, :, 1, dx].T
                    wk[conv, pair, g, dx, 64:128, :] = blk[:, :, 2, dx].T
    wall = np.concatenate([
        wp.reshape(24, 128, 64).transpose(1, 0, 2).reshape(128, 24 * 64),
        wk.reshape(24, 128, 64).transpose(1, 0, 2).reshape(128, 24 * 64),
    ], axis=1)
    return wall.astype(ml_dtypes.bfloat16)


def _expand_mask_full(mask):
    """mask [N,4,7,7] -> [N,256,56,56] nearest-upsampled, channel-repeated."""
    m = np.repeat(np.repeat(mask, 8, axis=2), 8, axis=3)
    return np.repeat(m, CIN // G, axis=1)


def _pack_mask_rows(mask_core):
    """mask [IMG,4,7,7] -> [128, IMG*PAIRS*7*PW] bf16 (padded cols zero)."""
    mexp = np.repeat(mask_core, 8, axis=-1)         # [IMG,4,7,56]
    mrow = np.zeros([IMG, PAIRS, 128, 7, PW], np.float32)
    for pair in range(PAIRS):
        for g in range(2):
            gg = 2 * pair + g
            mrow[:, pair, 64 * g:64 * (g + 1), :, 1:57] = mexp[:, gg][:, None]
    mrow = mrow.reshape(IMG * PAIRS, 128, 7 * PW).transpose(1, 0, 2)
    return mrow.reshape(128, IMG * PAIRS * 7 * PW).astype(ml_dtypes.bfloat16)


def _pack_xcc(xm_core):
    """xm [IMG,256,56,56] (masked, f32) -> [IMG,PAIRS,128,2*PADN] bf16 dup."""
    xp = np.zeros([IMG, CIN, PH, PW], np.float32)
    xp[:, :, 1:57, 1:57] = xm_core
    out = np.zeros([IMG, G, 128, PH, PW], np.float32)
    for g in range(G):
        blk = xp[:, 64 * g:64 * (g + 1)]            # [IMG,64,PH,PW]
        out[:, g, 0:64] = blk
        out[:, g, 64:128, 0:PH - 1] = blk[:, :, 1:PH]   # shifted up one row
    out = out.reshape(IMG, PAIRS, 2, 128, PADN).transpose(0, 1, 3, 2, 4)
    return np.ascontiguousarray(out.reshape(IMG, PAIRS, 128, 2 * PADN)
                                ).astype(ml_dtypes.bfloat16)


def make_in_maps(x, mask, w1, gamma1, beta1, w2, gamma2, beta2):
    x = np.asarray(x, np.float32)
    mask = np.asarray(mask, np.float32)
    bf = ml_dtypes.bfloat16
    xm_full = x * _expand_mask_full(mask)
    wall = _pack_weights(np.asarray(w1, np.float32), np.asarray(w2, np.float32))
    gb = np.zeros([128, 8], np.float32)
    for pair in range(PAIRS):
        sl = slice(128 * pair, 128 * (pair + 1))
        gb[:, 0 + pair] = np.asarray(gamma1, np.float32)[sl]
        gb[:, 2 + pair] = np.asarray(beta1, np.float32)[sl]
        gb[:, 4 + pair] = np.asarray(gamma2, np.float32)[sl]
        gb[:, 6 + pair] = np.asarray(beta2, np.float32)[sl]

    in_maps = []
    for core in range(N_CORES):
        sl = slice(IMG * core, IMG * (core + 1))
        in_maps.append({
            "xcc": _pack_xcc(xm_full[sl]),
            "xres": np.ascontiguousarray(
                x[sl].astype(bf).reshape(IMG, PAIRS, 128, H * W)),
            "wall": wall,
            "mrow": _pack_mask_rows(mask[sl]),
            "gb": gb,
        })
    return in_maps


def kernel(**inputs):
    if "nc" not in _prog_cache:
        _prog_cache["nc"] = _build_program()
    nc = _prog_cache["nc"]
    in_maps = make_in_maps(**inputs)
    res = run_bass_kernel_spmd(nc, in_maps, list(range(N_CORES)))
    y = np.concatenate(
        [res.results[i]["y"].reshape(IMG, CIN, H, W) for i in range(N_CORES)],
        axis=0)
    return y.astype(np.float32)


# revision 35
# speedup vs baseline: 1.0660x; 1.0660x over previous
"""Trainium2 Bass kernel for masked BasicBlock (grouped conv3x3 -> BN -> ReLU
-> masked grouped conv3x3 -> BN -> +residual -> ReLU).

Strategy: data-parallel over batch across 8 NeuronCores (2 images/core).
Grouped conv mapped to accumulating matmuls over a zero-padded SBUF image
layout with a row-duplicated ("dup") input so one K=128 matmul covers two ky
taps; the third ky row runs as K=64 matmuls on PE row-groups 2-3 read from the
shifted partition half.  Per 8-row output tile (N=448): 6 matmul "slots", each
two column-concurrent M=64 matmuls (g0 in PE cols 0:64, g1 in 64:128).

BN training-mode stats: hardware bn_stats per evacuated tile, bn_aggr per
core, then one 2KB AllReduce per conv (two warmup AllReduces absorb the
collectives-firmware bootstrap).

Conv2's masked input is built on-device: DVE computes m2*relu(a1*c1+b1) into a
padded tile, and the dup layout is produced by a DRAM round-trip (store m2,
re-load each group's two row-shifted halves).  Residual loads are pinned to
conv1 completion so they fill the collectives wait.  Output is written bf16
and widened to f32 on host.

Self-contained: hardcodes shapes from the problem spec.
"""
from contextlib import ExitStack

import numpy as np
import ml_dtypes

import concourse.bacc as bacc
import concourse.bass as bass
import concourse.mybir as mybir
from concourse.tile import TileContext
from concourse.bass_utils import run_bass_kernel_spmd

F32 = mybir.dt.float32
BF16 = mybir.dt.bfloat16
AF = mybir.ActivationFunctionType
ALU = mybir.AluOpType

N_CORES = 8
IMG = 2              # images per core
CIN = 256
G = 4
PAIRS = 2            # pairs of channel groups (128 ch each)
H = W = 56
PH, PW = 59, 58      # padded rows / cols (rows 0,57,58 and cols 0,57 zero)
PADN = PH * PW       # 3422
INT0 = PW            # flat offset of padded row 1
INTN = 56 * PW       # 3248: rows 1..56, all 58 cols
ROWT = 7             # 8-row output tiles per image
TN = 8 * W           # 448 pixels per psum tile
EPS = 1e-5
N_TOT = 16 * H * W
HNW = H * W // 2     # 1568

_prog_cache = {}


def _sub_ap(base, off, dims):
    """Custom free-dim access pattern on an existing AP (keeps partition dim)."""
    return bass.AP(
        tensor=base.tensor,
        offset=base.offset + off,
        ap=[list(base.ap[0])] + [list(d) for d in dims],
    )


def _widx(conv, pair, g, dx):
    return ((conv * PAIRS + pair) * 2 + g) * 3 + dx


def _build_program():
    nc = bacc.Bacc(num_devices=N_CORES)

    # conv1 input: host-premasked, padded, row-dup layout; g0|g1 concat per pair
    xcc_d = nc.dram_tensor("xcc", [IMG, PAIRS, 128, 2 * PADN], BF16,
                           kind="ExternalInput")
    xr_d = nc.dram_tensor("xres", [IMG, PAIRS, 128, H * W], BF16,
                          kind="ExternalInput")
    y_d = nc.dram_tensor("y", [IMG, PAIRS, 128, H * W], BF16,
                         kind="ExternalOutput")
    # all conv weights: 24 pair-tap lhsT [128,64] then 24 ky2 lhsT (rows 64:128)
    wall_d = nc.dram_tensor("wall", [128, 48 * 64], BF16, kind="ExternalInput")
    mr_d = nc.dram_tensor("mrow", [128, IMG * PAIRS * 7 * PW], BF16,
                          kind="ExternalInput")
    gb_d = nc.dram_tensor("gb", [128, 8], F32, kind="ExternalInput")

    with TileContext(nc) as tc, ExitStack() as es:
        consts = es.enter_context(tc.tile_pool(name="consts", bufs=1))
        small = es.enter_context(tc.tile_pool(name="small", bufs=16))
        ccp = es.enter_context(tc.tile_pool(name="ccp", bufs=4))
        c2p = es.enter_context(tc.tile_pool(name="c2p", bufs=4))
        m2p = es.enter_context(tc.tile_pool(name="m2p", bufs=4))
        xrp = es.enter_context(tc.tile_pool(name="xrp", bufs=4))
        yp = es.enter_context(tc.tile_pool(name="yp", bufs=4))
        scrp = es.enter_context(tc.tile_pool(name="scrp", bufs=2))
        psp = es.enter_context(tc.tile_pool(name="psp", bufs=8, space="PSUM"))
        drp = es.enter_context(tc.tile_pool(name="drp", bufs=1, space="DRAM"))

        # ---- collectives firmware warmup AllReduce (tiny, issued first) ----
        ccw_in = drp.tile([128, 4], F32, tag="ccwin", name="ccwin")
        ccw_out = drp.tile([128, 4], F32, addr_space="Shared",
                           tag="ccwout", name="ccwout")
        ccw2_in = drp.tile([128, 4], F32, tag="ccw2in", name="ccw2in")
        ccw2_out = drp.tile([128, 4], F32, addr_space="Shared",
                            tag="ccw2out", name="ccw2out")
        warm = small.tile([128, 4], F32, tag="warm", name="warm")
        nc.vector.memset(warm[:], 0.0)
        nc.sync.dma_start(out=ccw_in[:], in_=warm[:])
        nc.sync.dma_start(out=ccw2_in[:], in_=warm[:])
        for wi, wo in ((ccw_in, ccw_out), (ccw2_in, ccw2_out)):
            nc.gpsimd.collective_compute(
                "AllReduce", ALU.add,
                replica_groups=[list(range(N_CORES))],
                ins=[wi[:]], outs=[wo[:]],
            )

        cc_in = {c: drp.tile([128, 4], F32, tag=f"ccin{c}", name=f"ccin{c}")
                 for c in range(2)}
        cc_out = {c: drp.tile([128, 4], F32, addr_space="Shared",
                              tag=f"ccout{c}", name=f"ccout{c}") for c in range(2)}

        # ---- constants to SBUF (batched DMAs) ----
        wall_sb = consts.tile([128, 48 * 64], BF16, tag="wall", name="wall")
        nc.sync.dma_start(out=wall_sb[:], in_=wall_d[:])
        mr_sb = consts.tile([128, IMG * PAIRS * 7 * PW], BF16, tag="mr", name="mr")
        nc.sync.dma_start(out=mr_sb[:], in_=mr_d[:])
        gb_sb = consts.tile([128, 8], F32, tag="gb", name="gb")
        nc.sync.dma_start(out=gb_sb[:], in_=gb_d[:])
        eps_sb = consts.tile([128, 1], F32, tag="eps", name="eps")
        nc.vector.memset(eps_sb[:], EPS)

        def wp_ap(conv, pair, g, dx):      # pair-tap lhsT [128, 64]
            i = _widx(conv, pair, g, dx) * 64
            return wall_sb[:, i:i + 64]

        def wk_ap(conv, pair, g, dx):      # ky2 lhsT [128, 64], rows 0:64 zero
            i = (24 + _widx(conv, pair, g, dx)) * 64
            return wall_sb[:, i:i + 64]

        craw = {}
        for pair in range(PAIRS):
            for img in range(IMG):
                craw[(pair, img)] = consts.tile(
                    [128, H * W], BF16, tag=f"cr{pair}{img}", name=f"cr{pair}{img}")

        # bn_stats output groups: 6 values per (img, tile)
        st = {(c, p): consts.tile([128, IMG * ROWT * 6], F32,
                                  tag=f"st{c}{p}", name=f"st{c}{p}")
              for c in range(2) for p in range(PAIRS)}
        a_sb = {c: consts.tile([128, PAIRS], F32, tag=f"a{c}", name=f"a{c}")
                for c in range(2)}
        b_sb = {c: consts.tile([128, PAIRS], F32, tag=f"b{c}", name=f"b{c}")
                for c in range(2)}

        # ---------------- matmul block for one (conv, img, pair) ----------------
        def mm_block(conv, img, pair, rhs_tile):
            """rhs_tile(g) -> (tile, base_off) giving the dup layout source."""
            for t in range(ROWT):
                # full-bank pitch (512 f32) so partition offsets decompose
                # exactly in the accumulation-group bookkeeping
                ps = psp.tile([128, 512], F32, tag="ps", name="ps")
                # open the accumulation group for the whole bank with a tiny
                # N=1 matmul into the spare column (~50ns, M=128 start)
                tile0, off0 = rhs_tile(0)
                nc.tensor.matmul(
                    ps[:, TN:TN + 1], wall_sb[:, 0:128],
                    _sub_ap(tile0[:], off0, [[1, 1]]),
                    start=True, stop=False)
                for dx in range(3):
                    for g in range(2):
                        tile_g, off_g = rhs_tile(g)
                        rhs = _sub_ap(tile_g[:], off_g + (8 * t) * PW + dx,
                                      [[PW, 8], [1, W]])
                        nc.tensor.matmul(
                            ps[64 * g:64 * (g + 1), 0:TN],
                            wp_ap(conv, pair, g, dx), rhs,
                            start=False, stop=False)
                for dx in range(3):
                    for g in range(2):
                        tile_g, off_g = rhs_tile(g)
                        rhs = _sub_ap(tile_g[:], off_g + (8 * t + 1) * PW + dx,
                                      [[PW, 8], [1, W]])
                        nc.tensor.matmul(
                            ps[64 * g:64 * (g + 1), 0:TN],
                            wk_ap(conv, pair, g, dx), rhs,
                            start=False, stop=False)
                # close the group across all 128 partitions (tiny M=128 N=1)
                nc.tensor.matmul(
                    ps[:, TN:TN + 1], wall_sb[:, 0:128],
                    _sub_ap(tile0[:], off0, [[1, 1]]),
                    start=False, stop=True)
                # evacuate and take per-tile BN stats (baseline-proven path)
                seg = craw[(pair, img)][:, TN * t:TN * (t + 1)]
                col = img * ROWT + t
                nc.scalar.activation(out=seg, in_=ps[:, 0:TN], func=AF.Copy)
                nc.vector.bn_stats(
                    out=st[(conv, pair)][:, 6 * col:6 * (col + 1)], in_=seg)

        # ---------------- global BN stats -> a, b ----------------
        N_CORE_CNT = IMG * H * W

        def bn_coeffs(conv):
            sq = small.tile([128, 4], F32, tag=f"sq{conv}", name=f"sq{conv}")
            for pair in range(PAIRS):
                mv = small.tile([128, 2], F32, tag="mv", name="mv")
                nc.vector.bn_aggr(
                    out=mv[:],
                    in_=st[(conv, pair)][:].rearrange("p (n s) -> p n s", s=6))
                nc.vector.tensor_scalar_mul(
                    sq[:, 2 * pair:2 * pair + 1], mv[:, 0:1], float(N_CORE_CNT))
                msq0 = small.tile([128, 1], F32, tag="msq0", name="msq0")
                nc.vector.tensor_mul(msq0[:], mv[:, 0:1], mv[:, 0:1])
                nc.vector.tensor_add(msq0[:], msq0[:], mv[:, 1:2])
                nc.vector.tensor_scalar_mul(
                    sq[:, 2 * pair + 1:2 * pair + 2], msq0[:], float(N_CORE_CNT))
            nc.sync.dma_start(out=cc_in[conv][:], in_=sq[:])
            nc.gpsimd.collective_compute(
                "AllReduce", ALU.add,
                replica_groups=[list(range(N_CORES))],
                ins=[cc_in[conv][:]], outs=[cc_out[conv][:]],
            )
            sq2 = small.tile([128, 4], F32, tag=f"sq2{conv}", name=f"sq2{conv}")
            nc.sync.dma_start(out=sq2[:], in_=cc_out[conv][:])
            # batched over pairs: columns 0,2 are sums; 1,3 sum-squares
            mu = small.tile([128, PAIRS], F32, tag="mu", name="mu")
            nc.vector.tensor_scalar(
                out=mu[:], in0=_sub_ap(sq2[:], 0, [[2, PAIRS]]),
                scalar1=1.0 / N_TOT, scalar2=None, op0=ALU.mult)
            var = small.tile([128, PAIRS], F32, tag="var", name="var")
            nc.vector.tensor_scalar(
                out=var[:], in0=_sub_ap(sq2[:], 1, [[2, PAIRS]]),
                scalar1=1.0 / N_TOT, scalar2=None, op0=ALU.mult)
            msq = small.tile([128, PAIRS], F32, tag="msq", name="msq")
            nc.vector.tensor_mul(msq[:], mu[:], mu[:])
            nc.vector.tensor_sub(var[:], var[:], msq[:])       # biased var
            sd = small.tile([128, PAIRS], F32, tag="sd", name="sd")
            nc.scalar.activation(out=sd[:], in_=var[:], func=AF.Sqrt,
                                 bias=eps_sb[:])
            rstd = small.tile([128, PAIRS], F32, tag="rstd", name="rstd")
            nc.vector.reciprocal(out=rstd[:], in_=sd[:])
            gam = gb_sb[:, 4 * conv:4 * conv + 2]
            bet = gb_sb[:, 4 * conv + 2:4 * conv + 4]
            nc.vector.tensor_mul(a_sb[conv][:], gam, rstd[:])
            t3 = small.tile([128, PAIRS], F32, tag="t3", name="t3")
            nc.vector.tensor_mul(t3[:], a_sb[conv][:], mu[:])
            nc.vector.tensor_sub(b_sb[conv][:], bet, t3[:])

        # ---------------- conv1 ----------------
        cc_tiles = {}
        for img in range(IMG):
            for pair in range(PAIRS):
                cc = ccp.tile([128, 2 * PADN], BF16, tag="cc", name="cc")
                nc.sync.dma_start(out=cc[:], in_=xcc_d[img, pair])
                cc_tiles[(img, pair)] = cc
        for img in range(IMG):
            for pair in range(PAIRS):
                cc = cc_tiles[(img, pair)]
                mm_block(0, img, pair, lambda g, cc=cc: (cc, g * PADN))

        bn_coeffs(0)

        # ---------------- conv2 ----------------
        xr_tiles = {}
        for img in range(IMG):
            for pair in range(PAIRS):
                xr_tiles[(img, pair)] = xrp.tile([128, H * W], BF16,
                                                 tag="xr", name="xr")

        def conv2_block(img, pair):
            m2 = m2p.tile([128, PADN], BF16, tag="m2", name="m2")
            # zero borders: row 0, rows 57-58, cols 0 and 57 of rows 1-56
            nc.vector.memset(m2[:, 0:PW], 0)
            nc.vector.memset(m2[:, 57 * PW:PADN], 0)
            nc.vector.memset(_sub_ap(m2[:], PW, [[PW, 56], [1, 1]]), 0)
            nc.vector.memset(_sub_ap(m2[:], PW + 57, [[PW, 56], [1, 1]]), 0)
            # interior: m2 = relu(a1*c1 + b1) * mask  (two DVE ops)
            nc.vector.tensor_scalar(
                out=_sub_ap(m2[:], PW + 1, [[PW, 56], [1, 56]]),
                in0=craw[(pair, img)][:],
                scalar1=a_sb[0][:, pair:pair + 1],
                scalar2=b_sb[0][:, pair:pair + 1],
                op0=ALU.mult, op1=ALU.add)
            nc.vector.tensor_scalar(
                out=m2[:, INT0:INT0 + INTN], in0=m2[:, INT0:INT0 + INTN],
                scalar1=0.0, scalar2=None, op0=ALU.max)
            mask_ap = _sub_ap(mr_sb[:], (img * PAIRS + pair) * 7 * PW,
                              [[PW, 7], [0, 8], [1, PW]])
            nc.vector.tensor_mul(m2[:, INT0:INT0 + INTN],
                                 m2[:, INT0:INT0 + INTN], mask_ap)
            # dup layout via direct SBUF->SBUF DMA (skips the DRAM store leg
            # and its completion wait on the post-AllReduce critical path)
            c2 = {}
            for g in range(2):
                c = c2p.tile([128, PADN], BF16, tag="c2", name="c2")
                nc.sync.dma_start(out=c[0:64, 0:PADN],
                                  in_=m2[64 * g:64 * (g + 1), :])
                nc.sync.dma_start(out=c[64:128, 0:PADN - PW],
                                  in_=m2[64 * g:64 * (g + 1), PW:PADN])
                c2[g] = c
            mm_block(1, img, pair, lambda g, c2=c2: (c2[g], 0))

        # residual loads: pinned to conv1 completion (sequencing write makes
        # the DMA wait for conv1's last bn_stats) so they fill the
        # collectives-bootstrap gap instead of contending with conv2 prep
        for img in range(IMG):
            for pair in range(PAIRS):
                xr = xr_tiles[(img, pair)]
                nc.vector.tensor_copy(out=xr[0:1, 0:1],
                                      in_=st[(0, 1)][0:1, 0:1])
                nc.sync.dma_start(out=xr[:], in_=xr_d[img, pair])

        conv2_block(0, 0)
        conv2_block(0, 1)
        conv2_block(1, 0)
        conv2_block(1, 1)

        bn_coeffs(1)

        # ---------------- final: relu(a2*c2 + b2 + x) -> y (bf16) ----------------
        for img in range(IMG):
            for pair in range(PAIRS):
                for half in range(2):
                    seg = slice(HNW * half, HNW * (half + 1))
                    yt = yp.tile([128, HNW], BF16, tag="yt", name="yt")
                    # u = a2*c2 + x on DVE; relu(u + b2) on the idle ACT
                    # engine so the two stages pipeline across chunks
                    nc.vector.scalar_tensor_tensor(
                        out=yt[:], in0=craw[(pair, img)][:, seg],
                        scalar=a_sb[1][:, pair:pair + 1],
                        in1=xr_tiles[(img, pair)][:, seg],
                        op0=ALU.mult, op1=ALU.add)
                    nc.scalar.activation(
                        out=yt[:], in_=yt[:], func=AF.Relu,
                        bias=b_sb[1][:, pair:pair + 1])
                    nc.sync.dma_start(
                        out=_sub_ap(y_d[img, pair], HNW * half, [[1, HNW]]),
                        in_=yt[:])

    nc.compile()
    return nc


def _pack_weights(w1, w2):
    """w [256,64,3,3] f32 x2 -> wall [128, 48*64] bf16."""
    wp = np.zeros([2, PAIRS, 2, 3, 128, 64], np.float32)
    wk = np.zeros([2, PAIRS, 2, 3, 128, 64], np.float32)
    for conv, w in enumerate([w1, w2]):
        for pair in range(PAIRS):
            for g in range(2):
                blk = w[64 * (2 * pair + g):64 * (2 * pair + g + 1)]
                for dx in range(3):
                    wp[conv, pair, g, dx, 0:64, :] = blk[:, :, 0, dx].T
                    wp[conv, pair, g, dx, 64:128, :] = blk[:, :, 1, dx].T
                    wk[conv, pair, g, dx, 64:128, :] = blk[:, :, 2, dx].T
    wall = np.concatenate([
        wp.reshape(24, 128, 64).transpose(1, 0, 2).reshape(128, 24 * 64),
        wk.reshape(24, 128, 64).transpose(1, 0, 2).reshape(128, 24 * 64),
    ], axis=1)
    return wall.astype(ml_dtypes.bfloat16)


def _expand_mask_full(mask):
    """mask [N,4,7,7] -> [N,256,56,56] nearest-upsampled, channel-repeated."""
    m = np.repeat(np.repeat(mask, 8, axis=2), 8, axis=3)
    return np.repeat(m, CIN // G, axis=1)


def _pack_mask_rows(mask_core):
    """mask [IMG,4,7,7] -> [128, IMG*PAIRS*7*PW] bf16 (padded cols zero)."""
    mexp = np.repeat(mask_core, 8, axis=-1)         # [IMG,4,7,56]
    mrow = np.zeros([IMG, PAIRS, 128, 7, PW], np.float32)
    for pair in range(PAIRS):
        for g in range(2):
            gg = 2 * pair + g
            mrow[:, pair, 64 * g:64 * (g + 1), :, 1:57] = mexp[:, gg][:, None]
    mrow = mrow.reshape(IMG * PAIRS, 128, 7 * PW).transpose(1, 0, 2)
    return mrow.reshape(128, IMG * PAIRS * 7 * PW).astype(ml_dtypes.bfloat16)


def _pack_xcc(xm_core):
    """xm [IMG,256,56,56] (masked, f32) -> [IMG,PAIRS,128,2*PADN] bf16 dup."""
    xp = np.zeros([IMG, CIN, PH, PW], np.float32)
    xp[:, :, 1:57, 1:57] = xm_core
    out = np.zeros([IMG, G, 128, PH, PW], np.float32)
    for g in range(G):
        blk = xp[:, 64 * g:64 * (g + 1)]            # [IMG,64,PH,PW]
        out[:, g, 0:64] = blk
        out[:, g, 64:128, 0:PH - 1] = blk[:, :, 1:PH]   # shifted up one row
    out = out.reshape(IMG, PAIRS, 2, 128, PADN).transpose(0, 1, 3, 2, 4)
    return np.ascontiguousarray(out.reshape(IMG, PAIRS, 128, 2 * PADN)
                                ).astype(ml_dtypes.bfloat16)


def make_in_maps(x, mask, w1, gamma1, beta1, w2, gamma2, beta2):
    x = np.asarray(x, np.float32)
    mask = np.asarray(mask, np.float32)
    bf = ml_dtypes.bfloat16
    xm_full = x * _expand_mask_full(mask)
    wall = _pack_weights(np.asarray(w1, np.float32), np.asarray(w2, np.float32))
    gb = np.zeros([128, 8], np.float32)
    for pair in range(PAIRS):
        sl = slice(128 * pair, 128 * (pair + 1))
        gb[:, 0 + pair] = np.asarray(gamma1, np.float32)[sl]
        gb[:, 2 + pair] = np.asarray(beta1, np.float32)[sl]
        gb[:, 4 + pair] = np.asarray(gamma2, np.float32)[sl]
        gb[:, 6 + pair] = np.asarray(beta2, np.float32)[sl]

    in_maps = []
    for core in range(N_CORES):
        sl = slice(IMG * core, IMG * (core + 1))
        in_maps.append({
            "xcc": _pack_xcc(xm_full[sl]),
            "xres": np.ascontiguousarray(
                x[sl].astype(bf).reshape(IMG, PAIRS, 128, H * W)),
            "wall": wall,
            "mrow": _pack_mask_rows(mask[sl]),
            "gb": gb,
        })
    return in_maps


def kernel(**inputs):
    if "nc" not in _prog_cache:
        _prog_cache["nc"] = _build_program()
    nc = _prog_cache["nc"]
    in_maps = make_in_maps(**inputs)
    res = run_bass_kernel_spmd(nc, in_maps, list(range(N_CORES)))
    y = np.concatenate(
        [res.results[i]["y"].reshape(IMG, CIN, H, W) for i in range(N_CORES)],
        axis=0)
    return y.astype(np.float32)


# revision 37
# speedup vs baseline: 1.1027x; 1.0344x over previous
"""Trainium2 Bass kernel for masked BasicBlock (grouped conv3x3 -> BN -> ReLU
-> masked grouped conv3x3 -> BN -> +residual -> ReLU).

Strategy: data-parallel over batch across 8 NeuronCores (2 images/core).
Grouped conv mapped to accumulating matmuls over a zero-padded SBUF image
layout with a row-duplicated ("dup") input so one K=128 matmul covers two ky
taps; the third ky row runs as K=64 matmuls on PE row-groups 2-3 read from the
shifted partition half.  Per 8-row output tile (N=448): 6 matmul "slots", each
two column-concurrent M=64 matmuls (g0 in PE cols 0:64, g1 in 64:128).

BN training-mode stats: hardware bn_stats per evacuated tile, bn_aggr per
core, then one 2KB AllReduce per conv (two warmup AllReduces absorb the
collectives-firmware bootstrap).

Conv2's masked input is built on-device: DVE computes m2*relu(a1*c1+b1) into a
padded tile, and the dup layout is produced by direct SBUF->SBUF DMAs (two
row-shifted half loads per group).  Residual loads are pinned to
conv1 completion so they fill the collectives wait.  Output is written bf16
and widened to f32 on host.

Self-contained: hardcodes shapes from the problem spec.
"""
from contextlib import ExitStack

import numpy as np
import ml_dtypes

import concourse.bacc as bacc
import concourse.bass as bass
import concourse.mybir as mybir
from concourse.tile import TileContext
from concourse.bass_utils import run_bass_kernel_spmd

F32 = mybir.dt.float32
BF16 = mybir.dt.bfloat16
AF = mybir.ActivationFunctionType
ALU = mybir.AluOpType

N_CORES = 8
IMG = 2              # images per core
CIN = 256
G = 4
PAIRS = 2            # pairs of channel groups (128 ch each)
H = W = 56
PH, PW = 59, 58      # padded rows / cols (rows 0,57,58 and cols 0,57 zero)
PADN = PH * PW       # 3422
INT0 = PW            # flat offset of padded row 1
INTN = 56 * PW       # 3248: rows 1..56, all 58 cols
ROWT = 7             # 8-row output tiles per image
TN = 8 * W           # 448 pixels per psum tile
EPS = 1e-5
N_TOT = 16 * H * W
HNW = H * W // 2     # 1568

_prog_cache = {}


def _sub_ap(base, off, dims):
    """Custom free-dim access pattern on an existing AP (keeps partition dim)."""
    return bass.AP(
        tensor=base.tensor,
        offset=base.offset + off,
        ap=[list(base.ap[0])] + [list(d) for d in dims],
    )


def _widx(conv, pair, g, dx):
    return ((conv * PAIRS + pair) * 2 + g) * 3 + dx


def _build_program():
    nc = bacc.Bacc(num_devices=N_CORES)

    # conv1 input: host-premasked, padded, row-dup layout; g0|g1 concat per pair
    xcc_d = nc.dram_tensor("xcc", [IMG, PAIRS, 128, 2 * PADN], BF16,
                           kind="ExternalInput")
    xr_d = nc.dram_tensor("xres", [IMG, PAIRS, 128, H * W], BF16,
                          kind="ExternalInput")
    y_d = nc.dram_tensor("y", [IMG, PAIRS, 128, H * W], BF16,
                         kind="ExternalOutput")
    # all conv weights: 24 pair-tap lhsT [128,64] then 24 ky2 lhsT (rows 64:128)
    wall_d = nc.dram_tensor("wall", [128, 48 * 64], BF16, kind="ExternalInput")
    mr_d = nc.dram_tensor("mrow", [128, IMG * PAIRS * 7 * PW], BF16,
                          kind="ExternalInput")
    gb_d = nc.dram_tensor("gb", [128, 8], F32, kind="ExternalInput")

    with TileContext(nc) as tc, ExitStack() as es:
        consts = es.enter_context(tc.tile_pool(name="consts", bufs=1))
        small = es.enter_context(tc.tile_pool(name="small", bufs=16))
        ccp = es.enter_context(tc.tile_pool(name="ccp", bufs=4))
        c2p = es.enter_context(tc.tile_pool(name="c2p", bufs=4))
        m2p = es.enter_context(tc.tile_pool(name="m2p", bufs=4))
        xrp = es.enter_context(tc.tile_pool(name="xrp", bufs=4))
        yp = es.enter_context(tc.tile_pool(name="yp", bufs=4))
        scrp = es.enter_context(tc.tile_pool(name="scrp", bufs=2))
        psp = es.enter_context(tc.tile_pool(name="psp", bufs=8, space="PSUM"))
        drp = es.enter_context(tc.tile_pool(name="drp", bufs=1, space="DRAM"))

        # ---- collectives firmware warmup AllReduce (tiny, issued first) ----
        ccw_in = drp.tile([128, 4], F32, tag="ccwin", name="ccwin")
        ccw_out = drp.tile([128, 4], F32, addr_space="Shared",
                           tag="ccwout", name="ccwout")
        ccw2_in = drp.tile([128, 4], F32, tag="ccw2in", name="ccw2in")
        ccw2_out = drp.tile([128, 4], F32, addr_space="Shared",
                            tag="ccw2out", name="ccw2out")
        warm = small.tile([128, 4], F32, tag="warm", name="warm")
        nc.vector.memset(warm[:], 0.0)
        nc.sync.dma_start(out=ccw_in[:], in_=warm[:])
        nc.sync.dma_start(out=ccw2_in[:], in_=warm[:])
        for wi, wo in ((ccw_in, ccw_out), (ccw2_in, ccw2_out)):
            nc.gpsimd.collective_compute(
                "AllReduce", ALU.add,
                replica_groups=[list(range(N_CORES))],
                ins=[wi[:]], outs=[wo[:]],
            )

        cc_in = {c: drp.tile([128, 4], F32, tag=f"ccin{c}", name=f"ccin{c}")
                 for c in range(2)}
        cc_out = {c: drp.tile([128, 4], F32, addr_space="Shared",
                              tag=f"ccout{c}", name=f"ccout{c}") for c in range(2)}

        # ---- constants to SBUF (batched DMAs) ----
        wall_sb = consts.tile([128, 48 * 64], BF16, tag="wall", name="wall")
        nc.sync.dma_start(out=wall_sb[:], in_=wall_d[:])
        mr_sb = consts.tile([128, IMG * PAIRS * 7 * PW], BF16, tag="mr", name="mr")
        nc.sync.dma_start(out=mr_sb[:], in_=mr_d[:])
        gb_sb = consts.tile([128, 8], F32, tag="gb", name="gb")
        nc.sync.dma_start(out=gb_sb[:], in_=gb_d[:])
        eps_sb = consts.tile([128, 1], F32, tag="eps", name="eps")
        nc.vector.memset(eps_sb[:], EPS)

        def wp_ap(conv, pair, g, dx):      # pair-tap lhsT [128, 64]
            i = _widx(conv, pair, g, dx) * 64
            return wall_sb[:, i:i + 64]

        def wk_ap(conv, pair, g, dx):      # ky2 lhsT [128, 64], rows 0:64 zero
            i = (24 + _widx(conv, pair, g, dx)) * 64
            return wall_sb[:, i:i + 64]

        craw = {}
        for pair in range(PAIRS):
            for img in range(IMG):
                craw[(pair, img)] = consts.tile(
                    [128, H * W], BF16, tag=f"cr{pair}{img}", name=f"cr{pair}{img}")

        # bn_stats output groups: 6 values per (img, tile)
        st = {(c, p): consts.tile([128, IMG * ROWT * 6], F32,
                                  tag=f"st{c}{p}", name=f"st{c}{p}")
              for c in range(2) for p in range(PAIRS)}
        a_sb = {c: consts.tile([128, PAIRS], F32, tag=f"a{c}", name=f"a{c}")
                for c in range(2)}
        b_sb = {c: consts.tile([128, PAIRS], F32, tag=f"b{c}", name=f"b{c}")
                for c in range(2)}

        # ---------------- matmul block for one (conv, img, pair) ----------------
        def mm_block(conv, img, pair, rhs_tile):
            """rhs_tile(g) -> (tile, base_off) giving the dup layout source."""
            for t in range(ROWT):
                ps = psp.tile([128, 512], F32, tag="ps", name="ps")
                # per-column-half accumulation groups: hardware start/stop
                # apply per written partition, so each half opens with its
                # dx=0 pair matmul and closes with its ky2 dx=2 matmul
                for dx in range(3):
                    for g in range(2):
                        tile_g, off_g = rhs_tile(g)
                        rhs = _sub_ap(tile_g[:], off_g + (8 * t) * PW + dx,
                                      [[PW, 8], [1, W]])
                        nc.tensor.matmul(
                            ps[64 * g:64 * (g + 1), 0:TN],
                            wp_ap(conv, pair, g, dx), rhs,
                            start=(dx == 0), stop=False,
                            skip_group_check=True)
                for dx in range(3):
                    for g in range(2):
                        tile_g, off_g = rhs_tile(g)
                        rhs = _sub_ap(tile_g[:], off_g + (8 * t + 1) * PW + dx,
                                      [[PW, 8], [1, W]])
                        nc.tensor.matmul(
                            ps[64 * g:64 * (g + 1), 0:TN],
                            wk_ap(conv, pair, g, dx), rhs,
                            start=False, stop=(dx == 2),
                            skip_group_check=True)
                # evacuate and take per-tile BN stats (baseline-proven path)
                seg = craw[(pair, img)][:, TN * t:TN * (t + 1)]
                col = img * ROWT + t
                nc.scalar.activation(out=seg, in_=ps[:, 0:TN], func=AF.Copy)
                nc.vector.bn_stats(
                    out=st[(conv, pair)][:, 6 * col:6 * (col + 1)], in_=seg)

        # ---------------- global BN stats -> a, b ----------------
        N_CORE_CNT = IMG * H * W

        def bn_coeffs(conv):
            sq = small.tile([128, 4], F32, tag=f"sq{conv}", name=f"sq{conv}")
            for pair in range(PAIRS):
                mv = small.tile([128, 2], F32, tag="mv", name="mv")
                nc.vector.bn_aggr(
                    out=mv[:],
                    in_=st[(conv, pair)][:].rearrange("p (n s) -> p n s", s=6))
                nc.vector.tensor_scalar_mul(
                    sq[:, 2 * pair:2 * pair + 1], mv[:, 0:1], float(N_CORE_CNT))
                msq0 = small.tile([128, 1], F32, tag="msq0", name="msq0")
                nc.vector.tensor_mul(msq0[:], mv[:, 0:1], mv[:, 0:1])
                nc.vector.tensor_add(msq0[:], msq0[:], mv[:, 1:2])
                nc.vector.tensor_scalar_mul(
                    sq[:, 2 * pair + 1:2 * pair + 2], msq0[:], float(N_CORE_CNT))
            nc.sync.dma_start(out=cc_in[conv][:], in_=sq[:])
            nc.gpsimd.collective_compute(
                "AllReduce", ALU.add,
                replica_groups=[list(range(N_CORES))],
                ins=[cc_in[conv][:]], outs=[cc_out[conv][:]],
            )
            sq2 = small.tile([128, 4], F32, tag=f"sq2{conv}", name=f"sq2{conv}")
            nc.sync.dma_start(out=sq2[:], in_=cc_out[conv][:])
            # batched over pairs: columns 0,2 are sums; 1,3 sum-squares
            mu = small.tile([128, PAIRS], F32, tag="mu", name="mu")
            nc.vector.tensor_scalar(
                out=mu[:], in0=_sub_ap(sq2[:], 0, [[2, PAIRS]]),
                scalar1=1.0 / N_TOT, scalar2=None, op0=ALU.mult)
            var = small.tile([128, PAIRS], F32, tag="var", name="var")
            nc.vector.tensor_scalar(
                out=var[:], in0=_sub_ap(sq2[:], 1, [[2, PAIRS]]),
                scalar1=1.0 / N_TOT, scalar2=None, op0=ALU.mult)
            msq = small.tile([128, PAIRS], F32, tag="msq", name="msq")
            nc.vector.tensor_mul(msq[:], mu[:], mu[:])
            nc.vector.tensor_sub(var[:], var[:], msq[:])       # biased var
            sd = small.tile([128, PAIRS], F32, tag="sd", name="sd")
            nc.scalar.activation(out=sd[:], in_=var[:], func=AF.Sqrt,
                                 bias=eps_sb[:])
            rstd = small.tile([128, PAIRS], F32, tag="rstd", name="rstd")
            nc.vector.reciprocal(out=rstd[:], in_=sd[:])
            gam = gb_sb[:, 4 * conv:4 * conv + 2]
            bet = gb_sb[:, 4 * conv + 2:4 * conv + 4]
            nc.vector.tensor_mul(a_sb[conv][:], gam, rstd[:])
            t3 = small.tile([128, PAIRS], F32, tag="t3", name="t3")
            nc.vector.tensor_mul(t3[:], a_sb[conv][:], mu[:])
            nc.vector.tensor_sub(b_sb[conv][:], bet, t3[:])

        # ---------------- conv1 ----------------
        cc_tiles = {}
        for img in range(IMG):
            for pair in range(PAIRS):
                cc = ccp.tile([128, 2 * PADN], BF16, tag="cc", name="cc")
                nc.sync.dma_start(out=cc[:], in_=xcc_d[img, pair])
                cc_tiles[(img, pair)] = cc
        for img in range(IMG):
            for pair in range(PAIRS):
                cc = cc_tiles[(img, pair)]
                mm_block(0, img, pair, lambda g, cc=cc: (cc, g * PADN))

        bn_coeffs(0)

        # ---------------- conv2 ----------------
        xr_tiles = {}
        for img in range(IMG):
            for pair in range(PAIRS):
                xr_tiles[(img, pair)] = xrp.tile([128, H * W], BF16,
                                                 tag="xr", name="xr")

        def conv2_block(img, pair):
            m2 = m2p.tile([128, PADN], BF16, tag="m2", name="m2")
            # zero borders: row 0, rows 57-58, cols 0 and 57 of rows 1-56
            nc.vector.memset(m2[:, 0:PW], 0)
            nc.vector.memset(m2[:, 57 * PW:PADN], 0)
            nc.vector.memset(_sub_ap(m2[:], PW, [[PW, 56], [1, 1]]), 0)
            nc.vector.memset(_sub_ap(m2[:], PW + 57, [[PW, 56], [1, 1]]), 0)
            # interior: m2 = relu(a1*c1 + b1) * mask  (two DVE ops)
            nc.vector.tensor_scalar(
                out=_sub_ap(m2[:], PW + 1, [[PW, 56], [1, 56]]),
                in0=craw[(pair, img)][:],
                scalar1=a_sb[0][:, pair:pair + 1],
                scalar2=b_sb[0][:, pair:pair + 1],
                op0=ALU.mult, op1=ALU.add)
            nc.vector.tensor_scalar(
                out=m2[:, INT0:INT0 + INTN], in0=m2[:, INT0:INT0 + INTN],
                scalar1=0.0, scalar2=None, op0=ALU.max)
            mask_ap = _sub_ap(mr_sb[:], (img * PAIRS + pair) * 7 * PW,
                              [[PW, 7], [0, 8], [1, PW]])
            nc.vector.tensor_mul(m2[:, INT0:INT0 + INTN],
                                 m2[:, INT0:INT0 + INTN], mask_ap)
            # dup layout via direct SBUF->SBUF DMA (skips the DRAM store leg
            # and its completion wait on the post-AllReduce critical path)
            c2 = {}
            for g in range(2):
                c = c2p.tile([128, PADN], BF16, tag="c2", name="c2")
                nc.sync.dma_start(out=c[0:64, 0:PADN],
                                  in_=m2[64 * g:64 * (g + 1), :])
                nc.sync.dma_start(out=c[64:128, 0:PADN - PW],
                                  in_=m2[64 * g:64 * (g + 1), PW:PADN])
                c2[g] = c
            mm_block(1, img, pair, lambda g, c2=c2: (c2[g], 0))

        # residual loads: pinned to conv1 completion (sequencing write makes
        # the DMA wait for conv1's last bn_stats) so they fill the
        # collectives-bootstrap gap instead of contending with conv2 prep
        for img in range(IMG):
            for pair in range(PAIRS):
                xr = xr_tiles[(img, pair)]
                nc.vector.tensor_copy(out=xr[0:1, 0:1],
                                      in_=st[(0, 1)][0:1, 0:1])
                nc.sync.dma_start(out=xr[:], in_=xr_d[img, pair])

        conv2_block(0, 0)
        conv2_block(0, 1)
        conv2_block(1, 0)
        conv2_block(1, 1)

        bn_coeffs(1)

        # ---------------- final: relu(a2*c2 + b2 + x) -> y (bf16) ----------------
        for img in range(IMG):
            for pair in range(PAIRS):
                for half in range(2):
                    seg = slice(HNW * half, HNW * (half + 1))
                    yt = yp.tile([128, HNW], BF16, tag="yt", name="yt")
                    # u = a2*c2 + x on DVE; relu(u + b2) on the idle ACT
                    # engine so the two stages pipeline across chunks
                    nc.vector.scalar_tensor_tensor(
                        out=yt[:], in0=craw[(pair, img)][:, seg],
                        scalar=a_sb[1][:, pair:pair + 1],
                        in1=xr_tiles[(img, pair)][:, seg],
                        op0=ALU.mult, op1=ALU.add)
                    nc.scalar.activation(
                        out=yt[:], in_=yt[:], func=AF.Relu,
                        bias=b_sb[1][:, pair:pair + 1])
                    nc.sync.dma_start(
                        out=_sub_ap(y_d[img, pair], HNW * half, [[1, HNW]]),
                        in_=yt[:])

    nc.compile()
    return nc


def _pack_weights(w1, w2):
    """w [256,64,3,3] f32 x2 -> wall [128, 48*64] bf16."""
    wp = np.zeros([2, PAIRS, 2, 3, 128, 64], np.float32)
    wk = np.zeros([2, PAIRS, 2, 3, 128, 64], np.float32)
    for conv, w in enumerate([w1, w2]):
        for pair in range(PAIRS):
            for g in range(2):
                blk = w[64 * (2 * pair + g):64 * (2 * pair + g + 1)]
                for dx in range(3):
                    wp[conv, pair, g, dx, 0:64, :] = blk[:, :, 0, dx].T
                    wp[conv, pair, g, dx, 64:128, :] = blk[:, :, 1, dx].T
                    wk[conv, pair, g, dx, 64:128, :] = blk[:, :, 2, dx].T
    wall = np.concatenate([
        wp.reshape(24, 128, 64).transpose(1, 0, 2).reshape(128, 24 * 64),
        wk.reshape(24, 128, 64).transpose(1, 0, 2).reshape(128, 24 * 64),
    ], axis=1)
    return wall.astype(ml_dtypes.bfloat16)


def _expand_mask_full(mask):
    """mask [N,4,7,7] -> [N,256,56,56] nearest-upsampled, channel-repeated."""
    m = np.repeat(np.repeat(mask, 8, axis=2), 8, axis=3)
    return np.repeat(m, CIN // G, axis=1)


def _pack_mask_rows(mask_core):
    """mask [IMG,4,7,7] -> [128, IMG*PAIRS*7*PW] bf16 (padded cols zero)."""
    mexp = np.repeat(mask_core, 8, axis=-1)         # [IMG,4,7,56]
    mrow = np.zeros([IMG, PAIRS, 128, 7, PW], np.float32)
    for pair in range(PAIRS):
        for g in range(2):
            gg = 2 * pair + g
            mrow[:, pair, 64 * g:64 * (g + 1), :, 1:57] = mexp[:, gg][:, None]
    mrow = mrow.reshape(IMG * PAIRS, 128, 7 * PW).transpose(1, 0, 2)
    return mrow.reshape(128, IMG * PAIRS * 7 * PW).astype(ml_dtypes.bfloat16)


def _pack_xcc(xm_core):
    """xm [IMG,256,56,56] (masked, f32) -> [IMG,PAIRS,128,2*PADN] bf16 dup."""
    xp = np.zeros([IMG, CIN, PH, PW], np.float32)
    xp[:, :, 1:57, 1:57] = xm_core
    out = np.zeros([IMG, G, 128, PH, PW], np.float32)
    for g in range(G):
        blk = xp[:, 64 * g:64 * (g + 1)]            # [IMG,64,PH,PW]
        out[:, g, 0:64] = blk
        out[:, g, 64:128, 0:PH - 1] = blk[:, :, 1:PH]   # shifted up one row
    out = out.reshape(IMG, PAIRS, 2, 128, PADN).transpose(0, 1, 3, 2, 4)
    return np.ascontiguousarray(out.reshape(IMG, PAIRS, 128, 2 * PADN)
                                ).astype(ml_dtypes.bfloat16)


def make_in_maps(x, mask, w1, gamma1, beta1, w2, gamma2, beta2):
    x = np.asarray(x, np.float32)
    mask = np.asarray(mask, np.float32)
    bf = ml_dtypes.bfloat16
    xm_full = x * _expand_mask_full(mask)
    wall = _pack_weights(np.asarray(w1, np.float32), np.asarray(w2, np.float32))
    gb = np.zeros([128, 8], np.float32)
    for pair in range(PAIRS):
        sl = slice(128 * pair, 128 * (pair + 1))
        gb[:, 0 + pair] = np.asarray(gamma1, np.float32)[sl]
        gb[:, 2 + pair] = np.asarray(beta1, np.float32)[sl]
        gb[:, 4 + pair] = np.asarray(gamma2, np.float32)[sl]
        gb[:, 6 + pair] = np.asarray(beta2, np.float32)[sl]

    in_maps = []
    for core in range(N_CORES):
        sl = slice(IMG * core, IMG * (core + 1))
        in_maps.append({
            "xcc": _pack_xcc(xm_full[sl]),
            "xres": np.ascontiguousarray(
                x[sl].astype(bf).reshape(IMG, PAIRS, 128, H * W)),
            "wall": wall,
            "mrow": _pack_mask_rows(mask[sl]),
            "gb": gb,
        })
    return in_maps


def kernel(**inputs):
    if "nc" not in _prog_cache:
        _prog_cache["nc"] = _build_program()
    nc = _prog_cache["nc"]
    in_maps = make_in_maps(**inputs)
    res = run_bass_kernel_spmd(nc, in_maps, list(range(N_CORES)))
    y = np.concatenate(
        [res.results[i]["y"].reshape(IMG, CIN, H, W) for i in range(N_CORES)],
        axis=0)
    return y.astype(np.float32)
